# revision 46
# baseline (speedup 1.0000x reference)
"""LinearAttention kernel for Trainium2, 8 NeuronCores, data-parallel over batch.

Reference (per batch, c=256 channels, n=4096 tokens):
  xn   = x / ||x||_c * g1 * 16
  qkv  = Wqkv @ xn            (q,k,v each [512, n])
  q    = softmax_d(q) / 8     (softmax over d=64 within each of 8 heads)
  k    = softmax_n(k)
  ctx_h = k_h @ v_h^T
  out  = Wout @ concat_h(ctx_h^T @ q_h) + bout
  out  = out / ||out||_c * g2 * 16

Sharding: 16 batches -> 8 cores x 2 batches. No collectives.

v5 design notes (on top of v4):
 - Unified PSUM rotation: one [128, 1024] tag (bufs=3, 6 banks) carries all
   stage-A/B matmul outputs (ssq-pairs, q-pairs, kv blocks, pd-pairs,
   po-pairs, epilogue); ctx holds one pinned [128, 1024] tile whose halves
   are the two batches' accumulators (reused by ssqo in stage B).
 - q-projection and q-denominator grouped in [128, 1024] pairs; prefix
   channel-sums for both batches share one slot: halves the Act exp/ln and
   DVE reciprocal instruction count and fixed overheads.
 - Output-norm y^2 computed on Act as Square(256*y) in fp8e4 (y is ~1e-2,
   so the 256x pre-scale keeps y^2 above the fp8 subnormal floor), and the
   channel sum is ONE DoubleRow matmul against the fp8 ones block; the
   out-norm ln compensates with scale 2^-24.
 - DMA order: the first four x tiles and the Q-weight half issue before the
   bulky KV weights; x tiles prefetched three tiles ahead; startup x^2 runs
   on DVE (idle then) instead of the serial Pool ladder, and all x^2 issue
   before any xn so the in-order Pool queue never head-of-line blocks.
 - Epilogue split front (kdinv/ctx_sb, issued early) / back (matmuls).
 - Output stored bf16, upcast on host.
"""

import numpy as np

import concourse.bass as bass
import concourse.tile as tile
from concourse import bacc, mybir
from concourse.bass_utils import run_bass_kernel_spmd

F32 = mybir.dt.float32
F32R = mybir.dt.float32r
BF16 = mybir.dt.bfloat16
F8 = mybir.dt.float8e4
AF = mybir.ActivationFunctionType
OP = mybir.AluOpType
DR = mybir.MatmulPerfMode.DoubleRow

B = 16          # total batches
BL = 2          # batches per core
C = 256         # in channels
HID = 512       # heads * dim_head
HEADS = 8
DH = 64
N = 4096        # tokens
TN = 512        # token tile
NT = N // TN    # 8 tiles per batch
NB = TN // 128  # 4 128-token blocks per tile
NVT = 6         # vt rotation depth

ACT_TABLE_LN_EXP = 6  # index of natural_log_exp_and_others in act_func_sets

# optional profiling hook: profile scripts set this to record section spans
SECTION = lambda nc, label: None


def build_kernel(with_bout: bool):
    nc = bacc.Bacc("TRN2", target_bir_lowering=False, debug=False, num_devices=8)

    x_d = nc.dram_tensor("x", [BL, C, N], BF16, kind="ExternalInput").ap()
    wq_d = nc.dram_tensor("wqkvT", [128, 2, 3 * HID], F32R, kind="ExternalInput").ap()
    wo_d = nc.dram_tensor("woutTb", [64, HEADS, C], F32R, kind="ExternalInput").ap()
    g2_d = nc.dram_tensor("g2c", [128, 2], F32, kind="ExternalInput").ap()
    if with_bout:
        bo_d = nc.dram_tensor("boutc", [128, 2], F32, kind="ExternalInput").ap()
    o_d = nc.dram_tensor("out", [BL, C, N], BF16, kind="ExternalOutput").ap()

    xv = x_d.rearrange("b (cb p) n -> b p cb n", cb=2)
    ov = o_d.rearrange("b (cb p) n -> b p cb n", cb=2)

    with tile.TileContext(nc) as tc:
        with (
            tc.tile_pool(name="const", bufs=1) as const,
            tc.tile_pool(name="big", bufs=1) as big,
            tc.tile_pool(name="work", bufs=1) as work,
            tc.tile_pool(name="ps", bufs=1, space="PSUM") as ps,
        ):
            # one activation-table load for the whole program
            nc.scalar.add_instruction(mybir.InstLoadActFuncSet(
                name=nc.get_next_instruction_name(), ins=[], outs=[],
                act_func_set_id=ACT_TABLE_LN_EXP))

            # ---- cheap constants (engines idle anyway at t=0) ----
            ones8 = const.tile([128, 2, 128], F8)
            nc.gpsimd.memset(ones8, 1.0)
            bdb = const.tile([128, 128], BF16)
            nc.gpsimd.memset(bdb, 0.0)
            nc.gpsimd.memset(bdb[0:64, 0:64], 1.0)
            nc.gpsimd.memset(bdb[64:128, 64:128], 1.0)
            sclb = const.tile([1, 2], F32)
            nc.gpsimd.memset(sclb, 0.125)  # attention scale 1/8, via kdinv transpose
            onesb = const.tile([128, 128], BF16)
            nc.gpsimd.memset(onesb, 1.0)

            # ---- DMAs: first x tiles BEFORE the bulky weights ----
            xin_t = {}

            def dma_xin(j, bl):
                t = work.tile([128, 2, TN], BF16, tag="xin", bufs=4,
                              name=f"xin{j}_{bl}")
                nc.sync.dma_start(out=t, in_=xv[bl, :, :, j * TN:(j + 1) * TN])
                xin_t[(j, bl)] = t

            dma_xin(0, 0)
            dma_xin(0, 1)
            wqkvT = const.tile([128, 2, 3 * HID], F32R)
            nc.sync.dma_start(out=wqkvT[:, :, 0:HID], in_=wq_d[:, :, 0:HID])
            dma_xin(1, 0)
            dma_xin(1, 1)
            nc.sync.dma_start(out=wqkvT[:, :, HID:3 * HID],
                              in_=wq_d[:, :, HID:3 * HID])
            woutTb = const.tile([64, HEADS, C], F32R)
            nc.sync.dma_start(out=woutTb, in_=wo_d)
            g2c = const.tile([128, 2], F32)
            nc.sync.dma_start(out=g2c, in_=g2_d)
            g2c8 = const.tile([128, 2], F32)
            nc.vector.tensor_scalar_mul(out=g2c8, in0=g2c, scalar1=8.0)
            if with_bout:
                boutc = const.tile([128, 2], F32)
                nc.sync.dma_start(out=boutc, in_=bo_d)

            # ---- per-batch persistent tensors ----
            q_sm = [None] * BL
            ctxbig = ps.tile([128, 1024], F32, tag="ctx", bufs=1,
                             name="ctxbig")
            ctx_t = [ctxbig[:, 0:512], ctxbig[:, 512:1024]]
            nc.vector.memset(ctxbig, 0.0)
            for bl in range(BL):
                q_sm[bl] = big.tile([128, 4, N], BF16, tag="qsm", bufs=2,
                                    name=f"qsm{bl}")

            # ====== batch epilogue: W2 = (Wout @ ctx^T / kden / 8)^T ======
            w2T = [None] * BL

            def epilogue_front(bl):
                SECTION(nc, "epilogue")
                kdinv = work.tile([1, 512], F32, tag="kdi", bufs=2)
                nc.vector.reciprocal(out=kdinv, in_=ctx_t[bl][64:65, :])
                ctx_sb = work.tile([64, 512], F32R, tag="ctxsb", bufs=2)
                nc.scalar.activation(out=ctx_sb, in_=ctx_t[bl][0:64, :],
                                     func=AF.Copy)
                return kdinv, ctx_sb

            def epilogue(bl, front):
                SECTION(nc, "epilogue")
                kdinv, ctx_sb = front
                pkd = ps.tile([128, 512], F32, tag="mm", bufs=3)
                for h in range(HEADS):
                    nc.tensor.matmul(
                        pkd[0:64, 2 * h:2 * h + 2],
                        kdinv[0:1, h * 64:(h + 1) * 64],
                        sclb, start=True, stop=True,
                    )
                kdcol = work.tile([64, HEADS, 1], F32, tag="kdcol", bufs=2)
                pkd_v = pkd[0:64, 0:16].rearrange("p (h t) -> p h t", t=2)
                nc.vector.tensor_copy(out=kdcol, in_=pkd_v[:, :, 0:1])
                w2T[bl] = work.tile([128, 4, 256], BF16, tag="w2T", bufs=2,
                                    name=f"w2T{bl}")
                for hh in range(4):
                    pw2 = ps.tile([64, 2, 256], F32, tag="mm", bufs=3)
                    for i in range(2):
                        h = hh * 2 + i
                        nc.tensor.matmul(
                            pw2[:, i, :],
                            ctx_sb[:, h * 64:(h + 1) * 64],
                            woutTb[:, h, :],
                            start=True, stop=True,
                        )
                    for i in range(2):
                        h = hh * 2 + i
                        dst = w2T[bl][(h % 2) * 64:(h % 2) * 64 + 64, h // 2, :]
                        if i == 0:
                            nc.scalar.activation(out=dst, in_=pw2[:, i, :],
                                                 func=AF.Identity,
                                                 scale=kdcol[:, h, :])
                        else:
                            nc.vector.tensor_scalar_mul(
                                out=dst, in0=pw2[:, i, :],
                                scalar1=kdcol[:, h, :])

            # ========== stage A: norm-prefix software-pipelined 2 tiles ahead
            # phase a: x^2 for both batches into one shared channel-sum slot,
            # then ONE merged ln + ONE merged exp (the in-order Pool queue
            # never head-of-line blocks on the Act sinv chain); phase b: the
            # xn multiplies.
            def prefix_a(j, dve=False):
                SECTION(nc, "prefix")
                eng = nc.vector if dve else nc.gpsimd
                ssq = ps.tile([128, 1024], F32, tag="mm", bufs=3)
                for bl in range(BL):
                    if (j, bl) not in xin_t:
                        dma_xin(j, bl)
                    x2 = work.tile([128, 2, TN], F8, tag="x2", bufs=3)
                    eng.tensor_mul(x2, xin_t[(j, bl)], xin_t[(j, bl)])
                    nc.tensor.matmul(ssq[:, bl * 512:(bl + 1) * 512], ones8,
                                     x2, start=True, stop=True, perf_mode=DR)
                lns = work.tile([128, 1024], BF16, tag="lns", bufs=2)
                nc.scalar.activation(out=lns, in_=ssq, func=AF.Ln,
                                     scale=1.0 / 256.0)
                sinv = work.tile([128, 1024], F32, tag="sinv", bufs=2)
                nc.scalar.activation(out=sinv, in_=lns, func=AF.Exp,
                                     scale=-0.5)
                return sinv

            def prefix_b(j, bl, sinv, dve=False):
                SECTION(nc, "prefix")
                xin = xin_t.pop((j, bl))
                eng = nc.vector if dve else nc.gpsimd
                xn = work.tile([128, 2, TN], F32R, tag="xn", bufs=5)
                sv = sinv[:, bl * 512:(bl + 1) * 512]
                eng.tensor_mul(
                    xn, xin, sv.unsqueeze(1).broadcast_to([128, 2, TN]))
                return xn

            def body_a(j, bl, xn):
                t0 = j * TN
                eq = work.tile([128, 4, TN], BF16, tag="eq", bufs=3)
                qdi = work.tile([128, 4, TN], BF16, tag="qdi", bufs=3)

                # q = Wq @ xn in ob pairs -> one exp per [128, 1024] pair
                def q_pair(pp):
                    pq = ps.tile([128, 1024], F32, tag="mm", bufs=3)
                    for i in range(2):
                        ob = pp * 2 + i
                        for cb in range(2):
                            nc.tensor.matmul(
                                pq[:, i * 512:(i + 1) * 512],
                                wqkvT[:, cb, ob * 128:(ob + 1) * 128],
                                xn[:, cb, :],
                                start=(cb == 0), stop=(cb == 1),
                            )
                    nc.scalar.activation(
                        out=eq[:, 2 * pp:2 * pp + 2, :],
                        in_=pq.rearrange("p (i n) -> p i n", i=2),
                        func=AF.Exp)

                # softmax-d denominator pair -> one reciprocal per pair
                def pd_pair(pp):
                    pd = ps.tile([128, 1024], F32, tag="mm", bufs=3)
                    for i in range(2):
                        nc.tensor.matmul(pd[:, i * 512:(i + 1) * 512], bdb,
                                         eq[:, 2 * pp + i, :],
                                         start=True, stop=True)
                    with nc.allow_low_precision(reason="softmax recip bf16"):
                        nc.vector.reciprocal(
                            out=qdi[:, 2 * pp:2 * pp + 2, :],
                            in_=pd.rearrange("p (i n) -> p i n", i=2))

                def kv_block(nb):
                    pkv = ps.tile([128, 1024], F32, tag="mm", bufs=3)
                    for half in range(2):
                        for cb in range(2):
                            nc.tensor.matmul(
                                pkv[:, half * 512:(half + 1) * 512],
                                xn[:, cb, nb * 128:(nb + 1) * 128],
                                wqkvT[:, cb, HID + half * 512:
                                      HID + (half + 1) * 512],
                                start=(cb == 0), stop=(cb == 1),
                            )
                    ek = work.tile([128, HEADS, DH], BF16, tag="ek", bufs=6)
                    nc.scalar.activation(
                        out=ek.rearrange("p h d -> p (h d)"),
                        in_=pkv[:, 0:512], func=AF.Exp)
                    vt = work.tile([128, HEADS, 65], BF16, tag="vt", bufs=NVT)
                    nc.gpsimd.memset(vt[:, :, 64:65], 1.0)
                    vsrc = pkv[:, 512:1024].rearrange("p (h e) -> p h e", h=8)
                    nc.vector.tensor_copy(out=vt[:, :, 0:64], in_=vsrc)
                    return ek, vt

                def ctx_block(nb, ekvt):
                    ek, vt = ekvt
                    gnb = j * NB + nb
                    for h in range(HEADS):
                        nc.tensor.matmul(
                            ctx_t[bl][0:65, h * DH:(h + 1) * DH],
                            vt[:, h, :],
                            ek[:, h, :],
                            start=False, stop=(gnb == N // 128 - 1),
                            skip_group_check=True,
                        )

                SECTION(nc, "qpair")
                q_pair(0)
                q_pair(1)
                SECTION(nc, "kv01")
                kv01 = [kv_block(0), kv_block(1)]
                SECTION(nc, "kv23")
                kv23 = [kv_block(2), kv_block(3)]
                SECTION(nc, "pd0")
                pd_pair(0)
                SECTION(nc, "pd1")
                pd_pair(1)
                SECTION(nc, "ctx")
                ctx_block(0, kv01[0])
                ctx_block(1, kv01[1])
                ctx_block(2, kv23[0])
                ctx_block(3, kv23[1])
                SECTION(nc, "qsm")
                # q_sm = eq * qdi (16-bit packed -> 2x DVE mode)
                nc.vector.tensor_mul(q_sm[bl][:, :, t0:t0 + TN], eq, qdi)

            xns = {}
            for jj in range(2):
                sv = prefix_a(jj, dve=True)
                for bl in range(BL):
                    xns[(jj, bl)] = prefix_b(jj, bl, sv, dve=(jj == 0))
            for j in range(NT):
                if j + 3 < NT:
                    for bl in range(BL):
                        dma_xin(j + 3, bl)
                fronts = {}
                for bl in range(BL):
                    body_a(j, bl, xns.pop((j, bl)))
                    if j == NT - 1:
                        fronts[bl] = epilogue_front(bl)
                for bl in fronts:
                    epilogue(bl, fronts[bl])
                if j + 2 < NT:
                    sv = prefix_a(j + 2)
                    for bl in range(BL):
                        xns[(j + 2, bl)] = prefix_b(j + 2, bl, sv)

            # ================= stage B (tile-interleaved batches) ==========
            def po_pair(bl, t0, tn):
                SECTION(nc, "po")
                po = ps.tile([128, 1024], F32, tag="mm", bufs=3)
                for ob in range(2):
                    for kb in range(4):
                        nc.tensor.matmul(
                            po[:, ob * 512:ob * 512 + tn],
                            w2T[bl][:, kb, ob * 128:(ob + 1) * 128],
                            q_sm[bl][:, kb, t0:t0 + tn],
                            start=(kb == 0), stop=(kb == 3),
                        )
                return po

            def tail_v4(bl, t0, tn, po):
                SECTION(nc, "taily2")
                pov = po.rearrange("p (c n) -> p c n", c=2)[:, :, 0:tn]
                y2 = work.tile([128, 2, TN], F8, tag="y2", bufs=4)
                nc.scalar.activation(out=y2[:, :, 0:tn],
                                     in_=pov, func=AF.Square, scale=256.0)
                ssqo = ps.tile([128, 512], F32, tag="ctx", bufs=1,
                               name=f"sq{t0}_{bl}")
                nc.tensor.matmul(ssqo[:, 0:tn], ones8, y2[:, :, 0:tn],
                                 start=True, stop=True, perf_mode=DR)
                SECTION(nc, "tailrest")
                lno = work.tile([128, TN], F32, tag="lno", bufs=3, name="lno4")
                nc.scalar.activation(out=lno[:, 0:tn], in_=ssqo[:, 0:tn],
                                     func=AF.Ln, scale=2.0 ** -24)
                rgo = work.tile([128, TN], F32, tag="rgo", bufs=3, name="rgo4")
                nc.scalar.activation(out=rgo[:, 0:tn], in_=lno[:, 0:tn],
                                     func=AF.Exp, scale=-0.5)
                outt = work.tile([128, 2, TN], BF16, tag="outt", bufs=4)
                for cb in range(2):
                    nc.vector.scalar_tensor_tensor(
                        out=outt[:, cb, 0:tn],
                        in0=pov[:, cb, :],
                        scalar=g2c[:, cb:cb + 1],
                        in1=rgo[:, 0:tn],
                        op0=OP.mult, op1=OP.mult,
                    )
                nc.sync.dma_start(out=ov[bl, :, :, t0:t0 + tn],
                                  in_=outt[:, :, 0:tn])

            spans = [(j * TN, TN) for j in range(NT)]
            for t0, tn in spans:
                for bl in range(BL):
                    po = po_pair(bl, t0, tn)
                    tail_v4(bl, t0, tn, po)

    nc.finalize()
    return nc


_NC_CACHE = {}


def kernel(x, g1, Wqkv, Wout, bout, g2):
    x = np.ascontiguousarray(np.asarray(x, dtype=np.float32))
    g1 = np.asarray(g1, dtype=np.float32)
    Wqkv = np.asarray(Wqkv, dtype=np.float32)
    Wout = np.asarray(Wout, dtype=np.float32)
    bout = np.asarray(bout, dtype=np.float32)
    g2 = np.asarray(g2, dtype=np.float32)

    b, c, H, W = x.shape
    xr = x.reshape(b, c, H * W)

    bf = mybir.dt.np(BF16)
    # WqkvT [c, 3H] with g1 folded per channel, [p, cb, 3H] layout
    wqkvT = np.ascontiguousarray(
        (Wqkv.T * g1[:, None]).reshape(2, 128, 3 * HID).transpose(1, 0, 2)
    ).astype(np.float32)
    woutTb = np.ascontiguousarray(
        Wout.reshape(C, HEADS, DH).transpose(2, 1, 0)).astype(np.float32)
    g2c = np.ascontiguousarray(g2.reshape(2, 128).T)
    with_bout = bool(np.any(bout))

    if with_bout not in _NC_CACHE:
        _NC_CACHE[with_bout] = build_kernel(with_bout)
    nc = _NC_CACHE[with_bout]

    in_maps = []
    for core in range(8):
        m = {
            "x": np.ascontiguousarray(xr[core * BL:(core + 1) * BL].astype(bf)),
            "wqkvT": wqkvT, "woutTb": woutTb, "g2c": g2c,
        }
        if with_bout:
            m["boutc"] = np.ascontiguousarray(bout.reshape(2, 128).T)
        in_maps.append(m)
    res = run_bass_kernel_spmd(nc, in_maps, core_ids=list(range(8)))
    out = np.concatenate(
        [np.asarray(m["out"]).astype(np.float32) for m in res.results], axis=0)
    return out.reshape(b, c, H, W)


if __name__ == "__main__":
    rng = np.random.default_rng(0)
    inputs = dict(
        x=rng.standard_normal((16, 256, 64, 64), dtype=np.float32),
        g1=np.ones(256, np.float32),
        Wqkv=(rng.standard_normal((1536, 256), dtype=np.float32) * 256 ** -0.5),
        Wout=(rng.standard_normal((256, 512), dtype=np.float32) * 512 ** -0.5),
        bout=np.zeros(256, np.float32),
        g2=np.ones(256, np.float32),
    )
    out = kernel(**inputs)
    print("out", out.shape, out.dtype, np.abs(out).max())
